# revision 34
# baseline (speedup 1.0000x reference)
"""Trainium2 Bass kernel for nn_CurveEval3 (cubic B-spline curve evaluation).

Full inputs: ctrl_pts [4096,128,3] f32, knot_u [4096,132] f32.
Output: [4096,256,3] f32.

Sharding: pure data-parallel over batch; each of 8 cores handles 512 curves.

Scatter-scan algorithm (v2). Per block of 128 curves:
 1. NXT[j] = next distinct knot value after slice index j.
 2. t_first(row r) = index of the first eval point owned by span row r,
    computed arithmetically from NXT (the eval grid is uniform).
 3. A per-span table (6 knots, 6 guarded-recip dUs, marker, 12 bf16 ctrl
    values packed in 6 f32 words) is SCATTERED via indirect DMA to a DRAM
    grid buffer at row (curve*256 + t_first), one instruction per span row,
    dropping non-owner rows via the bounds check. Rows alternate over 3
    independent chains (separate DRAM buffers) so the tile framework's
    WAW serialization overlaps DMA-completion waits with SWDGE gen.
 4. Readback merges the 3 chains with accumulating DMAs; per-point channel
    values are reconstructed with tensor_tensor_scan forward-fills
    (state = (1-marker)*state + x), then the Cox-de Boor degree-3
    recursion and control-point combine run as [128 x 256] tile ops.
"""
import base64
import numpy as np

import concourse.bass as bass
import concourse.tile as tile
from concourse import mybir
from concourse.bass import AP, IndirectOffsetOnAxis
from concourse.bass_utils import run_bass_kernel_spmd

F32 = mybir.dt.float32
I32 = mybir.dt.int32
BF16 = mybir.dt.bfloat16

# The default walrus invocation runs with DGE (dynamic DMA) disabled, which
# silently mis-lowers indirect_dma_start. Inject the enable flag.
import concourse.bass_utils as _bu

if not getattr(_bu, "_dge_patch", False):
    _orig_run_command = _bu.run_command

    def _patched_run_command(argv, **kwargs):
        if argv and "walrus_driver" in str(argv[0]):
            argv = list(argv) + [
                "--dge-levels=scalar_dynamic_offset,vector_dynamic_offsets"
            ]
        return _orig_run_command(argv, **kwargs)

    _bu.run_command = _patched_run_command
    _bu._dge_patch = True

NCORES = 8
B_FULL = 4096
BC = B_FULL // NCORES      # 512 curves per core
M = 128                    # control points per curve
KW = 132                   # knot vector length
T = 256                    # eval points
DIM = 3
P = 3                      # degree
NBLK = BC // 128           # 4 blocks of 128 curves
TW = 144                   # padded NXT table width
NR = 125                   # scatterable span rows (sigma-3 in [0,124])
RCH = 20                   # f32 channels per scatter row (80 B)
NROW = 128 * T             # rows per scatter grid buffer
NCHAIN = 1                 # grid buffers (scatter waits are stripped)
EPS = 1e-8
U0 = 1e-5
DU = (1.0 - 2e-5) / (T - 1)

_U_B64 = (
    "rMUnN7zTgDvKqQA8t+lAPNGUgDzItKA8vtTAPLT04DxVigA9UJoQPUuqID1GujA9QspAPT3aUD046mA9M/pwPReFgD0VjYg9EpWQPRCdmD0NpaA9C62oPQi1sD0Gvbg9A8XAPQHNyD3+1NA9/NzYPfrk4D337Og99fTwPfL8+D14ggA+d4YEPnWKCD50jgw+c5IQPnKWFD5xmhg+b54cPm6iID5tpiQ+bKooPmquLD5psjA+aLY0Pme6OD5lvjw+ZMJAPmPGRD5iykg+Yc5MPl/SUD5e1lQ+XdpYPlzeXD5a4mA+WeZkPljqaD5X7mw+VvJwPlT2dD5T+ng+Uv58PiiBgD4og4I+J4WEPiaHhj4miYg+JYuKPiWNjD4kj44+I5GQPiOTkj4ilZQ+IpeWPiGZmD4gm5o+IJ2cPh+fnj4foaA+HqOiPh2lpD4dp6Y+HKmoPhurqj4braw+Gq+uPhqxsD4Zs7I+GLW0Phi3tj4Xubg+F7u6Pha9vD4Vv74+FcHAPhTDwj4TxcQ+E8fGPhLJyD4Sy8o+Ec3MPhDPzj4Q0dA+D9PSPg/V1D4O19Y+DdnYPg3b2j4M3dw+DN/ePgvh4D4K4+I+CuXkPgnn5j4I6eg+COvqPgft7D4H7+4+BvHwPgXz8j4F9fQ+BPf2PgT5+D4D+/o+Av38PgL//j6BgAA/gIEBP4CCAj+AgwM/f4QEP3+FBT9/hgY/focHP36ICD9+iQk/fYoKP32LCz99jAw/fY0NP3yODj98jw8/fJAQP3uRET97khI/e5MTP3qUFD96lRU/epYWP3qXFz95mBg/eZkZP3maGj94mxs/eJwcP3idHT93nh4/d58fP3egID92oSE/dqIiP3ajIz92pCQ/daUlP3WmJj91pyc/dKgoP3SpKT90qio/c6srP3OsLD9zrS0/cq4uP3KvLz9ysDA/crExP3GyMj9xszM/cbQ0P3C1NT9wtjY/cLc3P2+4OD9vuTk/b7o6P267Oz9uvDw/br09P26+Pj9tvz8/bcBAP23BQT9swkI/bMNDP2zERD9rxUU/a8ZGP2vHRz9qyEg/aslJP2rKSj9qy0s/acxMP2nNTT9pzk4/aM9PP2jQUD9o0VE/Z9JSP2fTUz9n1FQ/Z9VVP2bWVj9m11c/ZthYP2XZWT9l2lo/ZdtbP2TcXD9k3V0/ZN5eP2PfXz9j4GA/Y+FhP2PiYj9i42M/YuRkP2LlZT9h5mY/YednP2HoaD9g6Wk/YOpqP2Draz9f7Gw/X+1tP1/ubj9f728/XvBwP17xcT9e8nI/XfNzP130dD9d9XU/XPZ2P1z3dz9c+Hg/W/l5P1v6ej9b+3s/W/x8P1r9fT9a/n4/WP9/Pw=="
)
U_EXACT = np.frombuffer(base64.b64decode(_U_B64), dtype=np.float32).copy()  # [256]

# (k, r) pairs of the Cox-de Boor recursion, in order
PAIRS = [(1, 0), (2, 0), (2, 1), (3, 0), (3, 1), (3, 2)]

# t_first(r) = #{t : u_t <= NXT[r-1] + eps} computed as
# rne(clamp(NXT*INV_DU + CB_CONST, 0, 256)) with the -0.5 floor shift folded
# into the constant; slop-induced off-by-ones land within fp32 rounding of a
# knot where the spline's C^2 continuity makes the span choice irrelevant.
INV_DU = np.float32(1.0 / DU)
CB_CONST = np.float32((EPS - U0) / DU + 1.0 - 0.5)
OOB_BASE = 40000           # offset shift for non-owner rows -> bounds-dropped


def build_nc(split_waits=True, debug=False):
    nc = bass.Bass()
    knot = nc.dram_tensor("knot", [BC, KW], F32, kind="ExternalInput")
    ctrl = nc.dram_tensor("ctrl", [BC, M * DIM], F32, kind="ExternalInput")
    consts = nc.dram_tensor("consts", [T], F32, kind="ExternalInput")
    out = nc.dram_tensor("out", [BC, T * DIM], F32, kind="ExternalOutput")
    if debug:
        dbg_tf = nc.dram_tensor("dbg_tf", [128, 127], I32, kind="ExternalOutput")
        dbg_offs = nc.dram_tensor("dbg_offs", [128, NR], I32,
                                  kind="ExternalOutput")
        dbg_mbar = nc.dram_tensor("dbg_mbar", [128, T], F32,
                                  kind="ExternalOutput")
        dbg_f0 = nc.dram_tensor("dbg_f0", [128, T], F32, kind="ExternalOutput")
        dbg_cf0 = nc.dram_tensor("dbg_cf0", [128, T], F32, kind="ExternalOutput")
        dbg_ni = nc.dram_tensor("dbg_ni", [128, 4 * T], F32,
                                kind="ExternalOutput")
    # scatter grid buffers: [chain][block parity]; rows NROW..NROW+NR-1 are
    # per-span trash rows for non-owner spans (distinct rows so concurrent
    # descriptors never collide on one address)
    scat = [[nc.dram_tensor(f"scat_{c}_{q}", [NROW + NR, RCH], F32,
                            kind="Internal")
             for q in range(2)] for c in range(NCHAIN)]

    _strip_names = set()
    with tile.TileContext(nc) as tc:
        with tc.tile_pool(name="io", bufs=2) as io, \
             tc.tile_pool(name="big", bufs=2) as big, \
             tc.tile_pool(name="tmp", bufs=2) as tmp, \
             tc.tile_pool(name="glob", bufs=1) as glob:

            # u row broadcast tile (built once)
            u_row = glob.tile([128, T], F32, tag="u_row")
            nc.sync.dma_start(u_row[:, :], AP(consts, 0, [[0, 128], [1, T]]))
            ztile = glob.tile([128, T * RCH // 4], F32, tag="ztile")
            nc.vector.memset(ztile[:, :], 0.0)
            _fence_n = [0]

            def fence(ap_small):
                t_ = io.tile([128, 1], F32, tag=f"fj{_fence_n[0]}",
                             name=f"fj{_fence_n[0]}")
                _fence_n[0] += 1
                nc.vector.tensor_scalar(
                    out=t_[:, :], in0=ap_small, scalar1=0.0, scalar2=None,
                    op0=mybir.AluOpType.add)

            fence(u_row[:, 0:1])
            fence(ztile[:, 0:1])

            for blk in range(NBLK):
                r0 = blk * 128
                q = blk % 2
                # ---- load inputs --------------------------------------------
                K = io.tile([128, 136], F32, tag="K")
                nc.sync.dma_start(K[:, 0:KW], knot[r0:r0 + 128, :])
                # pad write doubles as the DMA fence on the DVE clock
                nc.vector.tensor_scalar(
                    out=K[:, KW:136], in0=K[:, 0:4], scalar1=0.0, scalar2=None,
                    op0=mybir.AluOpType.mult)
                CB = io.tile([128, 396], F32, tag="CB")
                nc.sync.dma_start(CB[:, 0:M * DIM], ctrl[r0:r0 + 128, :])
                nc.vector.tensor_scalar(
                    out=CB[:, M * DIM:396], in0=CB[:, 0:12], scalar1=0.0,
                    scalar2=None, op0=mybir.AluOpType.mult)

                # ---- NXT: next-distinct knot after slice idx j --------------
                KS = tmp.tile([128, 152], F32, tag="KS")
                nc.vector.memset(KS[:, :], 2.0)
                nc.vector.tensor_copy(KS[:, 0:129], K[:, 3:KW])
                m1 = tmp.tile([128, 148], mybir.dt.int8, tag="m1")
                nc.vector.tensor_tensor(
                    out=m1[:, :], in0=KS[:, 1:149], in1=KS[:, 0:148],
                    op=mybir.AluOpType.is_equal)
                nb = tmp.tile([128, 148], F32, tag="nb")
                nc.vector.tensor_copy(nb[:, :], KS[:, 1:149])
                nc.vector.copy_predicated(nb[:, :], m1[:, :], KS[:, 2:150])
                m2 = tmp.tile([128, TW], mybir.dt.int8, tag="m2")
                nc.vector.tensor_tensor(
                    out=m2[:, :], in0=nb[:, 0:TW], in1=KS[:, 0:TW],
                    op=mybir.AluOpType.is_equal)
                nxt = tmp.tile([128, TW], F32, tag="nxt")
                nc.vector.tensor_copy(nxt[:, :], nb[:, 0:TW])
                nc.vector.copy_predicated(nxt[:, :], m2[:, :], nb[:, 2:TW + 2])

                # ---- t_first + scatter offsets ------------------------------
                z = tmp.tile([128, 126], F32, tag="z")
                nc.vector.tensor_scalar(
                    out=z[:, :], in0=nxt[:, 0:126], scalar1=float(INV_DU),
                    scalar2=float(CB_CONST), op0=mybir.AluOpType.mult,
                    op1=mybir.AluOpType.add)
                nc.vector.tensor_scalar(
                    out=z[:, :], in0=z[:, :], scalar1=0.0, scalar2=256.0,
                    op0=mybir.AluOpType.max, op1=mybir.AluOpType.min)
                tf = tmp.tile([128, 127], I32, tag="tf")
                nc.vector.memset(tf[:, 0:1], 0)
                nc.vector.tensor_copy(tf[:, 1:127], z[:, :])
                # row 124 is the clamped owner of any points past tf[125]
                # (sigma-3 is clamped to <=124), so its validity must compare
                # against 256, not tf[125].
                nc.vector.memset(tf[:, 125:126], 256)
                valid = tmp.tile([128, NR], I32, tag="valid")
                nc.vector.tensor_tensor(
                    out=valid[:, :], in0=tf[:, 0:NR], in1=tf[:, 1:NR + 1],
                    op=mybir.AluOpType.is_lt)
                rowb = tmp.tile([128, NR], I32, tag="rowb")
                nc.gpsimd.iota(rowb[:, :], pattern=[[0, NR]], base=0,
                               channel_multiplier=T)
                o1 = tmp.tile([128, NR], I32, tag="o1")
                nc.vector.tensor_tensor(out=o1[:, :], in0=tf[:, 0:NR],
                                        in1=rowb[:, :], op=mybir.AluOpType.add)
                # offs = valid ? o1 : NROW + r (per-span trash row)
                tri = tmp.tile([128, NR], I32, tag="tri")
                nc.gpsimd.iota(tri[:, :], pattern=[[1, NR]], base=NROW,
                               channel_multiplier=0)
                offs = tmp.tile([128, NR], I32, tag="offs")
                nc.vector.tensor_tensor(
                    out=offs[:, :], in0=o1[:, :], in1=tri[:, :],
                    op=mybir.AluOpType.subtract)
                nc.vector.tensor_tensor(
                    out=offs[:, :], in0=offs[:, :], in1=valid[:, :],
                    op=mybir.AluOpType.mult)
                nc.vector.tensor_tensor(
                    out=offs[:, :], in0=offs[:, :], in1=tri[:, :],
                    op=mybir.AluOpType.add)
                if debug and blk == 0:
                    nc.sync.dma_start(dbg_tf[:, :], tf[:, :])
                    nc.sync.dma_start(dbg_offs[:, :], offs[:, :])

                # ---- per-span table ----------------------------------------
                gs = big.tile([128, NR, RCH], F32, tag="gs")
                for c in range(6):  # U[row+1+c]
                    nc.vector.tensor_copy(gs[:, :, c], K[:, 1 + c:NR + 1 + c])
                for p_i, (k, r) in enumerate(PAIRS):
                    dU = tmp.tile([128, NR], F32, tag="dU")
                    nc.vector.tensor_tensor(
                        out=dU[:, :], in0=K[:, 4 + r:NR + 4 + r],
                        in1=K[:, 4 - k + r:NR + 4 - k + r],
                        op=mybir.AluOpType.subtract)
                    gf = tmp.tile([128, NR], F32, tag="gf")
                    nc.vector.tensor_scalar(
                        out=gf[:, :], in0=dU[:, :], scalar1=0.0,
                        scalar2=None, op0=mybir.AluOpType.is_equal)
                    dU_ = tmp.tile([128, NR], F32, tag="dU_")
                    nc.vector.scalar_tensor_tensor(
                        out=dU_[:, :], in0=gf[:, :], scalar=1e-4, in1=dU[:, :],
                        op0=mybir.AluOpType.mult, op1=mybir.AluOpType.add)
                    rc = tmp.tile([128, NR], F32, tag="rc")
                    nc.vector.reciprocal(rc[:, :], dU_[:, :])
                    h = tmp.tile([128, NR], F32, tag="h")
                    nc.vector.tensor_scalar(
                        out=h[:, :], in0=gf[:, :], scalar1=-1.0, scalar2=1.0,
                        op0=mybir.AluOpType.mult, op1=mybir.AluOpType.add)
                    nc.vector.tensor_tensor(
                        out=gs[:, :, 6 + p_i], in0=rc[:, :], in1=h[:, :],
                        op=mybir.AluOpType.mult)
                # marker / pad via tensor_scalar (strided memset is untested
                # on this toolchain)
                nc.vector.tensor_scalar(
                    out=gs[:, :, 12], in0=K[:, 0:NR], scalar1=0.0, scalar2=1.0,
                    op0=mybir.AluOpType.mult, op1=mybir.AluOpType.add)
                nc.vector.tensor_scalar(
                    out=gs[:, :, 13], in0=K[:, 0:NR], scalar1=0.0, scalar2=None,
                    op0=mybir.AluOpType.mult)
                CB3 = CB[:, :].rearrange("p (m thr) -> p m thr", thr=DIM)
                gsb = gs[:, :, 14:20].bitcast(BF16)   # [128, NR, 12]
                for k in range(4):
                    for d in range(DIM):
                        nc.vector.tensor_copy(gsb[:, :, 3 * k + d],
                                              CB3[:, k:k + NR, d])

                # ---- zero-fill this parity's grid buffers -------------------
                ZW = T * RCH // 4
                for c in range(NCHAIN):
                    sview = scat[c][q][0:NROW, :].rearrange(
                        "(p t) c2 -> p (t c2)", p=128)
                    for zc in range(4):
                        # scalar-engine HWDGE: keeps the zero-fill off the
                        # sync queue, which is blocked behind the readback
                        nc.scalar.dma_start(
                            AP(sview.tensor, sview.offset + zc * ZW,
                               [sview.ap[0], [1, ZW]]),
                            ztile[:, :])

                # ---- scatter span rows at their first owned point ----------
                # Scatter target addresses are unique within a block (dedup'd
                # t_first; trash-row collisions are write-only garbage), so
                # only the first scatter per chain needs its hazard waits:
                # the in-order Pool queue + monotone sem thresholds cover the
                # rest. Later scatters' waits are stripped post-build (they
                # would otherwise serialize on each prior DMA's completion).
                for r in range(NR):
                    inst = nc.gpsimd.indirect_dma_start(
                        out=scat[r % NCHAIN][q][:, :],
                        out_offset=IndirectOffsetOnAxis(
                            ap=offs[:, r:r + 1], axis=0),
                        in_=gs[:, r, :], in_offset=None)
                    if r >= NCHAIN:
                        _strip_names.add(inst.ins.name)

                # ---- readback + merge chains -------------------------------
                rb = big.tile([128, T, RCH], F32, tag="rb")
                sview = scat[0][q][0:NROW, :].rearrange(
                    "(p t) c2 -> p t c2", p=128)
                nc.sync.dma_start(rb[:, :, :], sview)
                fence(rb[:, 0, 0:1])

                # ---- forward-fill scans ------------------------------------
                mbar = tmp.tile([128, T], F32, tag="mbar")
                nc.vector.tensor_scalar(
                    out=mbar[:, :], in0=rb[:, :, 12], scalar1=-1.0, scalar2=1.0,
                    op0=mybir.AluOpType.mult, op1=mybir.AluOpType.add)
                F = []
                for c in range(12):
                    f_ = tmp.tile([128, T], F32, tag=f"F{c}", name=f"F{c}")
                    nc.vector.tensor_tensor_scan(
                        out=f_[:, :], data0=mbar[:, :], data1=rb[:, :, c],
                        initial=0.0, op0=mybir.AluOpType.mult,
                        op1=mybir.AluOpType.add)
                    F.append(f_)
                CF = []
                for w in range(6):
                    f_ = tmp.tile([128, T], F32, tag=f"CF{w}", name=f"CF{w}")
                    nc.vector.tensor_tensor_scan(
                        out=f_[:, :], data0=mbar[:, :], data1=rb[:, :, 14 + w],
                        initial=0.0, op0=mybir.AluOpType.mult,
                        op1=mybir.AluOpType.add)
                    CF.append(f_)
                if debug and blk == 0:
                    nc.sync.dma_start(dbg_mbar[:, :], mbar[:, :])
                    nc.sync.dma_start(dbg_f0[:, :], F[0][:, :])
                    nc.sync.dma_start(dbg_cf0[:, :], CF[0][:, :])

                # ---- Cox-de Boor --------------------------------------------
                AU = []
                for c in range(3):
                    t_ = tmp.tile([128, T], F32, tag=f"AU{c}", name=f"AU{c}")
                    nc.vector.tensor_tensor(out=t_[:, :], in0=u_row[:, :],
                                            in1=F[c][:, :],
                                            op=mybir.AluOpType.subtract)
                    AU.append(t_)
                BU = []
                for r in range(3):
                    t_ = tmp.tile([128, T], F32, tag=f"BU{r}", name=f"BU{r}")
                    nc.vector.tensor_tensor(out=t_[:, :], in0=F[3 + r][:, :],
                                            in1=u_row[:, :],
                                            op=mybir.AluOpType.subtract)
                    BU.append(t_)
                Gp = []
                for p_i in range(6):
                    t_ = tmp.tile([128, T], F32, tag=f"Gp{p_i}", name=f"Gp{p_i}")
                    nc.vector.tensor_scalar(
                        out=t_[:, :], in0=F[6 + p_i][:, :], scalar1=0.0,
                        scalar2=None, op0=mybir.AluOpType.is_equal)
                    Gp.append(t_)

                temp = tmp.tile([128, T], F32, tag="temp")
                X = tmp.tile([128, T], F32, tag="X")
                Ni = [None, None, None, None]
                saved = None
                for k in range(1, P + 1):
                    saved = None  # symbolic zero at r=0
                    for r in range(k):
                        p_i = PAIRS.index((k, r))
                        # temp = Ni[r]*rdU' + g*1e-4
                        if k == 1 and r == 0:
                            nc.vector.scalar_tensor_tensor(
                                out=temp[:, :], in0=Gp[p_i][:, :], scalar=1e-4,
                                in1=F[6 + p_i][:, :],
                                op0=mybir.AluOpType.mult,
                                op1=mybir.AluOpType.add)
                        else:
                            nc.vector.tensor_tensor(
                                out=X[:, :], in0=Ni[r][:, :],
                                in1=F[6 + p_i][:, :], op=mybir.AluOpType.mult)
                            nc.vector.scalar_tensor_tensor(
                                out=temp[:, :], in0=Gp[p_i][:, :], scalar=1e-4,
                                in1=X[:, :], op0=mybir.AluOpType.mult,
                                op1=mybir.AluOpType.add)
                        ni_new = tmp.tile([128, T], F32, tag=f"Ni{r}_{k}",
                                          name=f"Ni{r}_{k}")
                        if saved is None:
                            nc.vector.tensor_tensor(
                                out=ni_new[:, :], in0=BU[r][:, :], in1=temp[:, :],
                                op=mybir.AluOpType.mult)
                        else:
                            nc.vector.tensor_tensor(
                                out=X[:, :], in0=BU[r][:, :], in1=temp[:, :],
                                op=mybir.AluOpType.mult)
                            nc.vector.tensor_tensor(
                                out=ni_new[:, :], in0=saved[:, :], in1=X[:, :],
                                op=mybir.AluOpType.add)
                        Ni[r] = ni_new
                        sv = tmp.tile([128, T], F32, tag=f"sv{k}_{r}",
                                      name=f"sv{k}_{r}")
                        nc.vector.tensor_tensor(
                            out=sv[:, :], in0=AU[3 - k + r][:, :], in1=temp[:, :],
                            op=mybir.AluOpType.mult)
                        saved = sv
                    Ni[k] = saved

                if debug and blk == 0:
                    for kk in range(4):
                        nc.sync.dma_start(dbg_ni[:, kk * T:(kk + 1) * T],
                                          Ni[kk][:, :])

                # ---- combine (ctrl read as packed bf16 halves) -------------
                out_s = big.tile([128, T, DIM], F32, tag="out_s")
                nc.vector.tensor_scalar(
                    out=out_s[:, 0, 0:1], in0=u_row[:, 0:1], scalar1=0.0,
                    scalar2=None, op0=mybir.AluOpType.mult)
                accd = tmp.tile([128, T], F32, tag="accd")

                def ctrl_ap(k, d):
                    p = 3 * k + d
                    w, h = p // 2, p % 2
                    cfb = CF[w][:, :].bitcast(BF16)   # [128, 512]
                    v = cfb[:, h:512]
                    return AP(v.tensor, v.offset, [v.ap[0], [2, T]])

                for d in range(DIM):
                    nc.vector.tensor_tensor(
                        out=accd[:, :], in0=Ni[0][:, :], in1=ctrl_ap(0, d),
                        op=mybir.AluOpType.mult)
                    for k in range(1, 4):
                        nc.vector.tensor_tensor(
                            out=X[:, :], in0=Ni[k][:, :], in1=ctrl_ap(k, d),
                            op=mybir.AluOpType.mult)
                        nc.vector.tensor_tensor(
                            out=accd[:, :], in0=accd[:, :], in1=X[:, :],
                            op=mybir.AluOpType.add)
                    nc.vector.tensor_copy(out_s[:, :, d], accd[:, :])
                nc.sync.dma_start(out[r0:r0 + 128, :], out_s[:, :, :])

    strip_waits(nc, _strip_names)
    if split_waits:
        split_multiwaits(nc)
    nc.finalize()
    return nc


def strip_waits(nc, names):
    """Remove sem waits from the marked scatter instructions (see the
    scatter-loop comment for the safety argument)."""
    for f in nc.m.functions:
        for bb in f.blocks:
            for inst in bb.instructions:
                if inst.name in names and inst.sync_info is not None:
                    inst.sync_info = mybir.SyncInfo(
                        on_wait=[], on_update=list(inst.sync_info.on_update))


def split_multiwaits(nc):
    """Walrus in this toolchain allows only one sync-wait per instruction.
    Split excess waits into preceding single-wait InstDrains."""
    split_n = 0
    for f in nc.m.functions:
        for bb in f.blocks:
            il = bb.instructions
            i = 0
            while i < len(il):
                inst = il[i]
                si = inst.sync_info
                if si is not None and len(si.on_wait) > 1:
                    for w in si.on_wait[:-1]:
                        d = mybir.InstDrain(
                            name=f"I-waitsplit-{split_n}",
                            ins=[], outs=[], bass_is_fusable=False)
                        split_n += 1
                        d.engine = inst.engine
                        d.sync_info = mybir.SyncInfo(on_wait=[w], on_update=[])
                        il.insert(i, d)
                        i += 1
                    inst.sync_info = mybir.SyncInfo(
                        on_wait=[si.on_wait[-1]], on_update=list(si.on_update))
                i += 1


_NC_CACHE = {}


def _get_nc():
    if "nc" not in _NC_CACHE:
        _NC_CACHE["nc"] = build_nc()
    return _NC_CACHE["nc"]


def _make_in_map(knot_slice: np.ndarray, ctrl_slice: np.ndarray) -> dict:
    return {
        "knot": knot_slice,
        "ctrl": ctrl_slice.reshape(BC, M * DIM),
        "consts": U_EXACT,
    }


def kernel(ctrl_pts: np.ndarray, knot_u: np.ndarray) -> np.ndarray:
    ctrl_pts = np.ascontiguousarray(ctrl_pts, dtype=np.float32)
    knot_u = np.ascontiguousarray(knot_u, dtype=np.float32)
    nc = _get_nc()
    in_maps = []
    for c in range(NCORES):
        sl = slice(c * BC, (c + 1) * BC)
        in_maps.append(_make_in_map(knot_u[sl], ctrl_pts[sl]))
    res = run_bass_kernel_spmd(nc, in_maps, core_ids=list(range(NCORES)))
    outs = [res.results[c]["out"].reshape(BC, T, DIM) for c in range(NCORES)]
    return np.concatenate(outs, axis=0)


if __name__ == "__main__":
    import reference
    import jax
    with jax.default_device(jax.devices("cpu")[0]):
        inputs = {k: np.asarray(v) for k, v in reference.setup_inputs().items()}
        expected = np.asarray(reference.reference(**{k: v for k, v in inputs.items()}))
    actual = kernel(**inputs)
    amax = np.abs(expected).max()
    err = np.abs(actual - expected).max()
    print("max abs err:", err, "rel:", err / amax)
